# revision 9
# baseline (speedup 1.0000x reference)
"""Trainium2 Bass kernel for nn_Aggregate (segment_reduce).

Reference computation:
    cat_idx = idx_inputs[:, argmax(softmax(cat_mask))]          # [N]
    agg     = segment_sum(inputs[:, 16:], cat_idx, 100000)       # [S, 128]
    out     = agg[cat_idx][:, top32(softmax(numeric_mask))] * conf

Strategy (no collectives needed):
  * Only the 32 top-k numeric columns survive to the output, and segment_sum
    is linear per column -> select those 32 columns FIRST (4x less data),
    and fold the conf scaling into them.
  * Sort rows by segment on the host.  After sorting, each segment's rows
    are one contiguous run; the segment total is the forward *segmented
    cumulative sum* (tensor_tensor_scan: state = m*state + x, m=0 at run
    starts) evaluated at the run's last row.
  * The device outputs a COMPACT per-segment table: a small ap_gather pulls
    the cumsum at each run-end position (one value per segment, ~10x fewer
    than rows).  The host then expands table -> rows with the same single
    fancy-index gather it would need to un-sort anyway.
  * Shard: the sorted rows are cut at segment boundaries into
    8 cores x 4 partition-groups x 8 windows = 256 independent chunks of
    ~3906 rows, each padded to K=4096.  Every chunk is a fully independent
    scan (initial=0), so nothing serializes across windows.  Layout per
    core: [128, F] with partition p = 32*group + col; window w of group g
    holds chunk (core*32 + g*8 + w) along free columns [w*K, (w+1)*K).

Everything data-dependent that the device needs (masks, run-end positions,
table slot layout) is precomputed on the host; the device graph is static.
"""

import sys
import types

import numpy as np

if "/opt/trn_rl_repo" not in sys.path:
    sys.path.insert(0, "/opt/trn_rl_repo")

import concourse.bacc as bacc
import concourse.mybir as mybir
import concourse.tile as tile
from concourse import library_config

# ----------------------------------------------------------------------------
# problem constants (hardcoded per spec)
N_ROWS = 1_000_000
NUM_CAT = 16
NUM_NUMERICS = 128
N_ARY = 32
NUM_SEGMENTS = 100_000

NCORES = 8
SEQ_PER_CORE = 4                    # partition-groups per core
NW = 16                             # windows (independent chunks) per group
NCHUNK = NCORES * SEQ_PER_CORE * NW  # 512 global chunks
CHUNK_NOMINAL = N_ROWS // NCHUNK    # 1953
P = 128
K = 2048                            # padded chunk length = window size
F = K * NW                          # 32768 free columns per core
SW = 256                            # table slots per window (max runs/chunk)
BX = K * 4                          # X bytes per window per partition
BM = K * 2                          # mask (bf16) bytes per window
BW = BX + BM                        # fused window bytes

import ml_dtypes
MASK_DT = ml_dtypes.bfloat16        # device mask dtype

_dt = mybir.dt

_CACHE: dict = {}


def _ensure_axon_hooks():
    """bass_utils imports antenv.axon_hooks for trace=True; provide a shim
    so the import never fails (hook stays None unless a profiler sets it)."""
    if "antenv.axon_hooks" in sys.modules:
        return sys.modules["antenv.axon_hooks"]
    mod = types.ModuleType("antenv.axon_hooks")
    hook = [None]
    mod.set_axon_ntff_profile_hook = lambda h: hook.__setitem__(0, h)
    mod.get_axon_ntff_profile_hook = lambda: hook[0]
    sys.modules["antenv.axon_hooks"] = mod
    return mod


def build_bass():
    """Build + compile the (SPMD, per-core identical) Bass graph once."""
    if "nc" in _CACHE:
        return _CACHE["nc"]
    nc = bacc.Bacc("TRN2", target_bir_lowering=False, debug=False,
                   num_devices=NCORES)
    xm_ext = nc.dram_tensor("xm", [P, NW * BW], _dt.uint8,
                            kind="ExternalInput").ap()
    idx_ext = nc.dram_tensor("idx", [P, NW * (SW // 16)], _dt.int16,
                             kind="ExternalInput").ap()
    out_ext = nc.dram_tensor("out", [P, NW * SW], _dt.float32,
                             kind="ExternalOutput").ap()

    with tile.TileContext(nc) as tc:
        with tc.tile_pool(name="xp", bufs=5) as xp, \
             tc.tile_pool(name="cp", bufs=8) as cp, \
             tc.tile_pool(name="op", bufs=16) as op, \
             tc.tile_pool(name="ip", bufs=1) as ip:
            nc.gpsimd.load_library(library_config.ap_gather)
            idx_sb = ip.tile([P, NW * (SW // 16)], _dt.int16, tag="idx")
            nc.gpsimd.dma_start(out=idx_sb[:], in_=idx_ext[:, :])
            for w in range(NW):
                ft = xp.tile([P, BW], _dt.uint8, tag="x")
                nc.sync.dma_start(out=ft[:], in_=xm_ext[:, w * BW:(w + 1) * BW])
                xt = ft[:, 0:BX].bitcast(_dt.float32)
                mt = ft[:, BX:BW].bitcast(_dt.bfloat16)
                ct = cp.tile([P, K], _dt.float32, tag="c")
                nc.vector.tensor_tensor_scan(
                    out=ct[:], data0=mt, data1=xt, initial=0.0,
                    op0=mybir.AluOpType.mult, op1=mybir.AluOpType.add,
                )
                ot = op.tile([P, SW], _dt.float32, tag="o")
                nc.gpsimd.ap_gather(
                    out_ap=ot[:], in_ap=ct[:],
                    idxs_ap=idx_sb[:, w * (SW // 16):(w + 1) * (SW // 16)],
                    channels=P, num_elems=K, d=1, num_idxs=SW,
                )
                nc.gpsimd.dma_start(out=out_ext[:, w * SW:(w + 1) * SW],
                                     in_=ot[:])
    nc.compile()
    _CACHE["nc"] = nc
    return nc


def _softmax64(v):
    v = np.asarray(v, dtype=np.float64)
    e = np.exp(v - v.max())
    return e / e.sum()


def prepare(inputs, idx_inputs, cat_mask, numeric_mask):
    """Host-side prep: top-k, column select + conf scale, sort, shard.

    Returns (in_maps, meta) where in_maps[i] feeds core i and meta carries
    what postprocess needs to expand the compact tables back to rows.
    """
    cat_mask = np.asarray(cat_mask)
    numeric_mask = np.asarray(numeric_mask)
    cm = _softmax64(cat_mask)
    ti = int(np.argmax(cm))                     # top_k(1) -> first max
    top_cat_val = cm[ti]
    nm = _softmax64(numeric_mask)
    order = np.argsort(-nm, kind="stable")[:N_ARY]   # descending, ties->low idx
    conf = ((nm[order] + top_cat_val) / 2.0).astype(np.float32)

    seg = np.ascontiguousarray(np.asarray(idx_inputs)[:, ti]).astype(np.int32)
    perm = np.argsort(seg, kind="stable")
    seg_s = seg[perm]

    inputs = np.asarray(inputs)
    sel = inputs[:, NUM_CAT + order].astype(np.float32) * conf[None, :]
    xs = sel[perm]                               # [N, 32] sorted by segment

    # run starts / per-row global slot rank (order of distinct segments)
    isstart = np.empty(N_ROWS, dtype=bool)
    isstart[0] = True
    isstart[1:] = seg_s[1:] != seg_s[:-1]
    rank_s = np.cumsum(isstart) - 1              # [N] slot of each row's run
    start_pos = np.flatnonzero(isstart)          # [R] run start row
    nruns = len(start_pos)
    end_pos = np.empty(nruns, dtype=np.int64)    # [R] run end row (inclusive)
    end_pos[:-1] = start_pos[1:] - 1
    end_pos[-1] = N_ROWS - 1

    # original-order slot of every row (for the final host gather)
    r_orig = np.empty(N_ROWS, dtype=np.int64)
    r_orig[perm] = rank_s

    # chunk cuts at run starts
    cut = np.empty(NCHUNK + 1, dtype=np.int64)
    cut[0], cut[NCHUNK] = 0, N_ROWS
    for j in range(1, NCHUNK):
        n = j * CHUNK_NOMINAL
        cut[j] = np.searchsorted(seg_s, seg_s[n], side="left")
    lens = np.diff(cut)
    assert lens.max() <= K, f"chunk too long: {lens.max()} > {K}"

    # first run of each chunk
    run_cut = np.searchsorted(start_pos, cut[:-1])
    run_cut = np.append(run_cut, nruns)
    n_per_chunk = np.diff(run_cut)
    assert n_per_chunk.max() <= SW, f"chunk runs overflow: {n_per_chunk.max()}"

    # mask: 1 where same segment as previous row
    m_all = np.ones(N_ROWS, dtype=MASK_DT)
    m_all[isstart] = 0

    in_maps = []
    for i in range(NCORES):
        X = np.zeros((P, F), dtype=np.float32)
        M = np.ones((P, F), dtype=MASK_DT)       # pad: m=1 keeps state flat
        # (packed into the fused byte stream below)
        IDX = np.zeros((P, NW * (SW // 16)), dtype=np.int16)
        for g in range(SEQ_PER_CORE):
            for w in range(NW):
                j = (i * SEQ_PER_CORE + g) * NW + w
                c0, c1 = int(cut[j]), int(cut[j + 1])
                ln = c1 - c0
                X[g * 32:(g + 1) * 32, w * K:w * K + ln] = xs[c0:c1].T
                M[g * 32:(g + 1) * 32, w * K:w * K + ln] = m_all[c0:c1][None, :]
                ends_w = end_pos[run_cut[j]:run_cut[j + 1]] - c0
                nw = len(ends_w)
                loc = np.zeros(SW, dtype=np.int16)
                loc[:nw] = ends_w.astype(np.int16)
                wrapped = loc.reshape(SW // 16, 16).T      # [16, SW/16]
                IDX[g * 32:g * 32 + 16,
                    w * (SW // 16):(w + 1) * (SW // 16)] = wrapped
                IDX[g * 32 + 16:g * 32 + 32,
                    w * (SW // 16):(w + 1) * (SW // 16)] = wrapped
        XM = np.concatenate(
            [X.view(np.uint8).reshape(P, NW, BX),
             np.ascontiguousarray(M).view(np.uint8).reshape(P, NW, BM)],
            axis=2).reshape(P, NW * BW)
        in_maps.append({"xm": np.ascontiguousarray(XM), "idx": IDX})
    meta = {"r_orig": r_orig, "n_per_chunk": n_per_chunk, "nruns": nruns}
    return in_maps, meta


def postprocess(results, meta):
    """Expand compact per-segment tables to the [N, 32] output."""
    n_per_chunk = meta["n_per_chunk"]
    table = np.empty((meta["nruns"], N_ARY), dtype=np.float32)
    pos = 0
    for i in range(NCORES):
        O = results[i]["out"].reshape(SEQ_PER_CORE, 32, NW, SW)
        for g in range(SEQ_PER_CORE):
            for w in range(NW):
                j = (i * SEQ_PER_CORE + g) * NW + w
                nw = int(n_per_chunk[j])
                if nw:
                    table[pos:pos + nw] = O[g, :, w, :nw].T
                    pos += nw
    assert pos == meta["nruns"]
    return table[meta["r_orig"]]


def run(in_maps, trace=False, trace_kwargs=None):
    _ensure_axon_hooks()
    from concourse.bass_utils import run_bass_kernel_spmd
    nc = build_bass()
    return run_bass_kernel_spmd(nc, in_maps, core_ids=list(range(NCORES)),
                                trace=trace, **(trace_kwargs or {}))


def kernel(inputs, idx_inputs, cat_mask, numeric_mask):
    in_maps, meta = prepare(inputs, idx_inputs, cat_mask, numeric_mask)
    res = run(in_maps, trace=False)
    return postprocess(res.results, meta)


# revision 11
# speedup vs baseline: 2.4345x; 2.4345x over previous
"""Trainium2 Bass kernel for nn_Aggregate (segment_reduce).

Reference computation:
    cat_idx = idx_inputs[:, argmax(softmax(cat_mask))]          # [N]
    agg     = segment_sum(inputs[:, 16:], cat_idx, 100000)       # [S, 128]
    out     = agg[cat_idx][:, top32(softmax(numeric_mask))] * conf

Strategy (no collectives, no gpsimd):
  * Only the 32 top-k numeric columns survive to the output, and segment_sum
    is linear per column -> select those 32 columns FIRST (4x less data),
    and fold the conf scaling into them.
  * Sort rows by segment on the host.  After sorting, each segment's rows
    are one contiguous run; the segment total is the forward *segmented
    cumulative sum* (tensor_tensor_scan: state = m*state + x, m=0 at run
    starts, fp32 state) evaluated at the run's last row.
  * The device streams in (x, m) in bf16, runs the scan in-place (the
    cumsum overwrites the mask region of the same SBUF tile), and streams
    the raw cumsum straight back out.  The HOST picks the run-end values
    out of the returned cumsum - it already does a full-size fancy-index
    gather to un-sort the output, so this adds no asymptotic host work.
  * Shard: the sorted rows are cut at segment boundaries into
    8 cores x 4 partition-groups x 16 windows = 512 independent chunks of
    ~1953 rows, each padded to K=2048.  Every chunk is a fully independent
    scan, and all 16 windows are SBUF-resident (no buffer reuse), so the
    only device-side dependencies are load_w -> scan_w -> store_w.

Everything data-dependent (masks, run-end positions, slot bookkeeping) is
precomputed on the host; the device graph is static.
"""

import sys
import types

import ml_dtypes
import numpy as np

if "/opt/trn_rl_repo" not in sys.path:
    sys.path.insert(0, "/opt/trn_rl_repo")

import concourse.bacc as bacc
import concourse.mybir as mybir
import concourse.tile as tile

# ----------------------------------------------------------------------------
# problem constants (hardcoded per spec)
N_ROWS = 1_000_000
NUM_CAT = 16
NUM_NUMERICS = 128
N_ARY = 32
NUM_SEGMENTS = 100_000

NCORES = 8
SEQ_PER_CORE = 4                     # partition-groups per core
NW = 16                              # windows (independent chunks) per group
NCHUNK = NCORES * SEQ_PER_CORE * NW  # 512 global chunks
CHUNK_NOMINAL = N_ROWS // NCHUNK     # 1953
P = 128
K = 2048                             # padded chunk length = window size
F = K * NW                           # 32768 free columns per core
BX = K * 2                           # X (bf16) bytes per window per partition
BM = K * 2                           # mask/cumsum (bf16) bytes per window
BW = BX + BM                         # fused window bytes

BF16 = ml_dtypes.bfloat16

_dt = mybir.dt

_CACHE: dict = {}


def _ensure_axon_hooks():
    """bass_utils imports antenv.axon_hooks for trace=True; provide a shim
    so the import never fails (hook stays None unless a profiler sets it)."""
    if "antenv.axon_hooks" in sys.modules:
        return sys.modules["antenv.axon_hooks"]
    mod = types.ModuleType("antenv.axon_hooks")
    hook = [None]
    mod.set_axon_ntff_profile_hook = lambda h: hook.__setitem__(0, h)
    mod.get_axon_ntff_profile_hook = lambda: hook[0]
    sys.modules["antenv.axon_hooks"] = mod
    return mod


def build_bass():
    """Build + compile the (SPMD, per-core identical) Bass graph once."""
    if "nc" in _CACHE:
        return _CACHE["nc"]
    nc = bacc.Bacc("TRN2", target_bir_lowering=False, debug=False,
                   num_devices=NCORES)
    xm_ext = nc.dram_tensor("xm", [P, NW * BW], _dt.uint8,
                            kind="ExternalInput").ap()
    out_ext = nc.dram_tensor("out", [P, NW * BM], _dt.uint8,
                             kind="ExternalOutput").ap()

    with tile.TileContext(nc) as tc:
        with tc.tile_pool(name="xp", bufs=1) as xp:
            for w in range(NW):
                ft = xp.tile([P, BW], _dt.uint8, tag=f"w{w}")
                nc.sync.dma_start(out=ft[:], in_=xm_ext[:, w * BW:(w + 1) * BW])
                xt = ft[:, 0:BX].bitcast(_dt.bfloat16)
                mc = ft[:, BX:BW].bitcast(_dt.bfloat16)
                # in-place: the cumsum lands where the mask was (1:1 bytes,
                # each element read before it is overwritten)
                nc.vector.tensor_tensor_scan(
                    out=mc, data0=mc, data1=xt, initial=0.0,
                    op0=mybir.AluOpType.mult, op1=mybir.AluOpType.add,
                )
                nc.scalar.dma_start(out=out_ext[:, w * BM:(w + 1) * BM],
                                    in_=ft[:, BX:BW])
    nc.compile()
    _CACHE["nc"] = nc
    return nc


def _softmax64(v):
    v = np.asarray(v, dtype=np.float64)
    e = np.exp(v - v.max())
    return e / e.sum()


def prepare(inputs, idx_inputs, cat_mask, numeric_mask):
    """Host-side prep: top-k, column select + conf scale, sort, shard.

    Returns (in_maps, meta): in_maps[i] feeds core i; meta carries the
    indices postprocess needs to pull run-end cumsums out of the returned
    tables and expand them to rows.
    """
    cat_mask = np.asarray(cat_mask)
    numeric_mask = np.asarray(numeric_mask)
    cm = _softmax64(cat_mask)
    ti = int(np.argmax(cm))                     # top_k(1) -> first max
    top_cat_val = cm[ti]
    nm = _softmax64(numeric_mask)
    order = np.argsort(-nm, kind="stable")[:N_ARY]   # descending, ties->low idx
    conf = ((nm[order] + top_cat_val) / 2.0).astype(np.float32)

    seg = np.ascontiguousarray(np.asarray(idx_inputs)[:, ti]).astype(np.int32)
    perm = np.argsort(seg, kind="stable")
    seg_s = seg[perm]

    inputs = np.asarray(inputs)
    sel = inputs[:, NUM_CAT + order].astype(np.float32) * conf[None, :]
    xs = sel[perm].astype(BF16)                  # [N, 32] sorted, bf16

    # run bookkeeping
    isstart = np.empty(N_ROWS, dtype=bool)
    isstart[0] = True
    isstart[1:] = seg_s[1:] != seg_s[:-1]
    rank_s = np.cumsum(isstart) - 1              # [N] slot of each row's run
    start_pos = np.flatnonzero(isstart)          # [R] run start row
    nruns = len(start_pos)
    end_pos = np.empty(nruns, dtype=np.int64)    # [R] run end row (inclusive)
    end_pos[:-1] = start_pos[1:] - 1
    end_pos[-1] = N_ROWS - 1

    # original-order slot of every row (for the final host gather)
    r_orig = np.empty(N_ROWS, dtype=np.int64)
    r_orig[perm] = rank_s

    # chunk cuts at run starts
    cut = np.empty(NCHUNK + 1, dtype=np.int64)
    cut[0], cut[NCHUNK] = 0, N_ROWS
    for j in range(1, NCHUNK):
        n = j * CHUNK_NOMINAL
        cut[j] = np.searchsorted(seg_s, seg_s[n], side="left")
    lens = np.diff(cut)
    assert lens.max() <= K, f"chunk too long: {lens.max()} > {K}"

    # first run of each chunk
    run_cut = np.searchsorted(start_pos, cut[:-1])
    run_cut = np.append(run_cut, nruns)

    # mask: 1 where same segment as previous row
    m_all = np.ones(N_ROWS, dtype=BF16)
    m_all[isstart] = 0

    in_maps = []
    core_gidx = []      # per core: (g_arr, off_arr) into the returned [4,32,F]
    for i in range(NCORES):
        X = np.zeros((P, F), dtype=BF16)
        M = np.ones((P, F), dtype=BF16)          # pad: m=1 keeps state flat
        gs, offs = [], []
        for g in range(SEQ_PER_CORE):
            for w in range(NW):
                j = (i * SEQ_PER_CORE + g) * NW + w
                c0, c1 = int(cut[j]), int(cut[j + 1])
                ln = c1 - c0
                X[g * 32:(g + 1) * 32, w * K:w * K + ln] = xs[c0:c1].T
                M[g * 32:(g + 1) * 32, w * K:w * K + ln] = m_all[c0:c1][None, :]
                ends = end_pos[run_cut[j]:run_cut[j + 1]] - c0   # chunk-local
                gs.append(np.full(len(ends), g, dtype=np.int64))
                offs.append(w * K + ends)
        core_gidx.append((np.concatenate(gs), np.concatenate(offs)))
        XM = np.concatenate(
            [X.view(np.uint8).reshape(P, NW, BX),
             M.view(np.uint8).reshape(P, NW, BM)],
            axis=2).reshape(P, NW * BW)
        in_maps.append({"xm": np.ascontiguousarray(XM)})
    meta = {"r_orig": r_orig, "core_gidx": core_gidx, "nruns": nruns}
    return in_maps, meta


def postprocess(results, meta):
    """Pull run-end cumsums from the returned tables, expand to rows."""
    table = np.empty((meta["nruns"], N_ARY), dtype=np.float32)
    pos = 0
    for i in range(NCORES):
        C = results[i]["out"].view(BF16).astype(np.float32)      # [P, F]
        C = C.reshape(SEQ_PER_CORE, 32, F)
        g_arr, off_arr = meta["core_gidx"][i]
        vals = C[g_arr, :, off_arr]                              # [R_i, 32]
        table[pos:pos + len(g_arr)] = vals
        pos += len(g_arr)
    assert pos == meta["nruns"]
    return table[meta["r_orig"]]


def run(in_maps, trace=False, trace_kwargs=None):
    _ensure_axon_hooks()
    from concourse.bass_utils import run_bass_kernel_spmd
    nc = build_bass()
    return run_bass_kernel_spmd(nc, in_maps, core_ids=list(range(NCORES)),
                                trace=trace, **(trace_kwargs or {}))


def kernel(inputs, idx_inputs, cat_mask, numeric_mask):
    in_maps, meta = prepare(inputs, idx_inputs, cat_mask, numeric_mask)
    res = run(in_maps, trace=False)
    return postprocess(res.results, meta)
